# revision 14
# baseline (speedup 1.0000x reference)
"""Trainium2 Bass kernel for nn_CNNCrossPatchBackbone (sparse cross-patch attention).

Strategy: 8 cores = 4 batches x {ctx self-attention, tgt cross-attention}.
Fully task-parallel, no collectives. Each core: 1024 q-tokens x 1024
kv-tokens, 16 heads of dim 64, D=1024.

v3.5 vs v3.3 (308.7us):
  * All 1-partition bias matmuls removed: V bias and out-proj bias applied
    by the DVE during PSUM evacuation (tensor_add against host-prebroadcast
    bias tiles). Saves ~10us of PE at 317ns per bias matmul.
  * Q-projection chains 1..7 moved INTO the phase-2 head loop (one chain
    every other head), filling the PE slack under the ACT exp floor; Q bias
    applied via DVE tensor_scalar_add (per-partition scalar). Shrinks the
    serial phase-1 region by ~24us.
  * Phase 2 per head: 8 uniform 1024-wide exp chunks (shared 3-buf psum),
    AV into two single-bank [65,512] psums (full-speed 216ns AV matmuls;
    the 2-bank [65,1024] variant ran at 334ns), denominator reciprocal on
    SBUF copy (custom DVE ops corrupt on PSUM input), GPS broadcast +
    mixed-dtype normalize-mul lag one head behind, off the critical path.

Measured engine floors per core: ACT exp 8.5us/head (phase-2 pacer), PE
~216ns/512-col matmul when uncoupled. Predicted ~275us.
"""

import sys

sys.path.insert(0, "/opt/trn_rl_repo")

import ml_dtypes
import numpy as np

import concourse.bass as bass  # noqa: F401
import concourse.tile as tile
from concourse import bacc, mybir
from concourse.bass_utils import run_bass_kernel_spmd

B, K, D, H = 4, 2048, 1024, 16
NCTX = K // 2
NTOK = 1024
HD = D // H  # 64
IMAGE_SIZE = 224.0
MAX_POS = 1024
P = 128
DT = D // P  # 8
TT = NTOK // P  # 8
F32 = mybir.dt.float32
BF16 = mybir.dt.bfloat16
BF = ml_dtypes.bfloat16


def build_nc():
    nc = bacc.Bacc("TRN2", target_bir_lowering=False, debug=False, num_devices=8)

    xqT_ext = nc.dram_tensor("xqT", [P, DT * NTOK], BF16, kind="ExternalInput")
    xkT_ext = nc.dram_tensor("xkT", [P, DT * NTOK], BF16, kind="ExternalInput")
    wq_ext = nc.dram_tensor("wq", [P, DT * D], BF16, kind="ExternalInput")
    wk_ext = nc.dram_tensor("wk", [P, DT * D], BF16, kind="ExternalInput")
    wv_ext = nc.dram_tensor("wv", [P, DT * D], BF16, kind="ExternalInput")
    wo_ext = nc.dram_tensor("wo", [P, DT * D], BF16, kind="ExternalInput")
    bvb_ext = nc.dram_tensor("bvb", [P, D], BF16, kind="ExternalInput")
    bob_ext = nc.dram_tensor("bob", [P, D], F32, kind="ExternalInput")
    biasqk_ext = nc.dram_tensor("biasqk", [P, 2 * DT], F32, kind="ExternalInput")
    out_ext = nc.dram_tensor("out", [NTOK, D], F32, kind="ExternalOutput")

    from contextlib import ExitStack

    with tile.TileContext(nc) as tc:
        es0 = ExitStack()   # whole-kernel sbuf pools
        psA = ExitStack()   # shared [128,1024]f32 psum (proj chains + S chunks)
        esW = ExitStack()   # wo staging (lives to end)
        esK = ExitStack()   # wv/xk/wq/xq staging (live through phase 2)
        es1a = ExitStack()  # wk staging (closes after K chains)
        es2 = ExitStack()   # phase-2 sbuf pools (open after K chains)
        es3 = ExitStack()   # phase-2 AV psum
        es4 = ExitStack()   # phase-3 pools

        cpool = es0.enter_context(tc.tile_pool(name="const", bufs=1))
        p_qt = es0.enter_context(tc.tile_pool(name="p_qt", bufs=3))
        p_spr = es0.enter_context(tc.tile_pool(name="p_spr", bufs=DT))
        p_va = es0.enter_context(tc.tile_pool(name="p_va", bufs=TT))
        p_ot = es0.enter_context(tc.tile_pool(name="p_ot", bufs=DT))

        ones_f = cpool.tile([P, P], F32)
        nc.gpsimd.memset(ones_f[:], 1.0)
        ones_bf = cpool.tile([P, P], BF16)
        nc.vector.tensor_copy(ones_bf[:], ones_f[:])
        biasT = cpool.tile([P, 2 * DT], F32)
        nc.sync.dma_start(biasT[:], biasqk_ext.ap())
        bvb = cpool.tile([P, D], BF16)
        nc.scalar.dma_start(bvb[:], bvb_ext.ap())
        bob = cpool.tile([P, D], F32)
        nc.scalar.dma_start(bob[:], bob_ext.ap())

        QT = {}  # chain idx -> [128, NTOK] bf16 tile
        # SPRP[c] holds K^T for head 2c in rows 0:64 and head 2c+1 in rows 64:128
        SPRP = [p_spr.tile([P, NTOK], BF16, tag="spr", name=f"spr{c}") for c in range(DT)]
        VA = [p_va.tile([P, H * (HD + 1)], BF16, tag="va", name=f"va{i}") for i in range(TT)]
        OT = [p_ot.tile([P, NTOK], BF16, tag="ot", name=f"ot{i}") for i in range(DT)]
        for va in VA:
            nc.vector.tensor_copy(
                va[:].rearrange("p (h c) -> p h c", c=HD + 1)[:, :, HD : HD + 1],
                ones_bf[:, 0:H].rearrange("p (h c) -> p h c", c=1),
            )

        # ---- staging + DMAs ----
        ps = psA.enter_context(tc.tile_pool(name="ps", bufs=3, space="PSUM"))
        p_wo = esW.enter_context(tc.tile_pool(name="p_wo", bufs=1))
        p_wv = esK.enter_context(tc.tile_pool(name="p_wv", bufs=1))
        p_xk = esK.enter_context(tc.tile_pool(name="p_xk", bufs=1))
        p_wq = esK.enter_context(tc.tile_pool(name="p_wq", bufs=1))
        p_xq = esK.enter_context(tc.tile_pool(name="p_xq", bufs=1))
        p_wk = es1a.enter_context(tc.tile_pool(name="p_wk", bufs=1))

        def stage(pool, n, tag):
            t = pool.tile([P, n * NTOK], BF16, tag=tag, name=tag)
            return t, [t[:, i * NTOK : (i + 1) * NTOK] for i in range(n)]

        # wk/xk split per-dt on two queues: K chains start as tiles land.
        wk_t, WK = stage(p_wk, DT, "wk")
        xk_t, XK = stage(p_xk, DT, "xk")
        for dt in range(DT):
            nc.sync.dma_start(
                wk_t[:, dt * NTOK : (dt + 1) * NTOK],
                wk_ext.ap()[:, dt * NTOK : (dt + 1) * NTOK],
            )
            nc.gpsimd.dma_start(
                xk_t[:, dt * NTOK : (dt + 1) * NTOK],
                xkT_ext.ap()[:, dt * NTOK : (dt + 1) * NTOK],
            )
        wq_t, WQ = stage(p_wq, DT, "wq")
        nc.sync.dma_start(wq_t[:], wq_ext.ap())
        xq_t, XQ = stage(p_xq, DT, "xq")
        nc.gpsimd.dma_start(xq_t[:], xqT_ext.ap())
        wv_t, WV = stage(p_wv, DT, "wv")
        nc.sync.dma_start(wv_t[:], wv_ext.ap())
        wo_t, WO = stage(p_wo, DT, "wo")
        nc.sync.dma_start(wo_t[:], wo_ext.ap())

        # ---- phase 1: K chains + Q chain 0 (ACT identity evac) ----
        for c in range(DT):
            kps = ps.tile([P, NTOK], F32, tag="ps", name=f"kps{c}")
            for nh in range(2):
                sl = slice(nh * 512, (nh + 1) * 512)
                for dt in range(DT):
                    nc.tensor.matmul(
                        kps[:, sl], WK[dt][:, c * P : (c + 1) * P], XK[dt][:, sl],
                        start=(dt == 0), stop=(dt == DT - 1),
                    )
            nc.scalar.activation(
                SPRP[c][0:HD, :], kps[0:HD, :],
                mybir.ActivationFunctionType.Identity,
                bias=biasT[0:HD, DT + c : DT + c + 1],
            )
            nc.scalar.activation(
                SPRP[c][HD:P, :], kps[HD:P, :],
                mybir.ActivationFunctionType.Identity,
                bias=biasT[HD:P, DT + c : DT + c + 1],
            )

        def emit_q_chain(c, evac_dve):
            qps = ps.tile([P, NTOK], F32, tag="ps", name=f"qps{c}")
            for nh in range(2):
                sl = slice(nh * 512, (nh + 1) * 512)
                for dt in range(DT):
                    nc.tensor.matmul(
                        qps[:, sl], WQ[dt][:, c * P : (c + 1) * P], XQ[dt][:, sl],
                        start=(dt == 0), stop=(dt == DT - 1),
                    )
            QT[c] = p_qt.tile([P, NTOK], BF16, tag="qt", name=f"qt{c}")
            if evac_dve:
                nc.vector.tensor_scalar_add(QT[c][:], qps[:], biasT[:, c : c + 1])
            else:
                nc.scalar.activation(
                    QT[c][:], qps[:],
                    mybir.ActivationFunctionType.Identity,
                    bias=biasT[:, c : c + 1],
                )

        emit_q_chain(0, evac_dve=False)
        es1a.close()  # wk staging done

        # ---- phase-2 sbuf pools ----
        p_a = es2.enter_context(tc.tile_pool(name="p_a", bufs=14))
        p_rdn = es2.enter_context(tc.tile_pool(name="p_rdn", bufs=1))
        p_rr = es2.enter_context(tc.tile_pool(name="p_rr", bufs=1))
        p_rb = es2.enter_context(tc.tile_pool(name="p_rb", bufs=2))
        p_os = es2.enter_context(tc.tile_pool(name="p_os", bufs=2))

        AT = {}  # (h, kc) -> a tile [128k, 1024q] bf16

        def emit_s_chunk(h, kc):
            qt, po = h // 2, (h % 2) * HD
            s_ps = ps.tile([P, NTOK], F32, tag="ps", name=f"s{h}_{kc}")
            for j in range(2):
                nc.tensor.matmul(
                    s_ps[:, j * 512 : (j + 1) * 512],
                    SPRP[h // 2][po : po + HD, kc * P : (kc + 1) * P],
                    QT[qt][po : po + HD, j * 512 : (j + 1) * 512],
                    start=True, stop=True,
                )
            a_t = p_a.tile([P, NTOK], BF16, tag="a", name=f"a{h}_{kc}")
            nc.scalar.activation(a_t[:], s_ps[:], mybir.ActivationFunctionType.Exp)
            AT[(h, kc)] = a_t

        # V chains (DVE bias-add evac) interleaved with S/exp of head 0
        for tt in range(TT):
            vps = ps.tile([P, NTOK], F32, tag="ps", name=f"vps{tt}")
            for nh in range(2):
                sl = slice(nh * 512, (nh + 1) * 512)
                for dt in range(DT):
                    nc.tensor.matmul(
                        vps[:, sl], XK[dt][:, tt * P : (tt + 1) * P],
                        WV[dt][:, sl],
                        start=(dt == 0), stop=(dt == DT - 1),
                    )
            nc.vector.tensor_add(
                VA[tt][:].rearrange("p (h c) -> p h c", c=HD + 1)[:, :, 0:HD],
                bvb[:].rearrange("p (h c) -> p h c", c=HD),
                vps[:].rearrange("p (h c) -> p h c", c=HD),
            )
            emit_s_chunk(0, tt)

        ps_o = es3.enter_context(tc.tile_pool(name="ps_o", bufs=2, space="PSUM"))

        # ---- phase 2 main loop: S(i+1) | AV(i) | Q-chain | norm(i) | mul(i-1)
        def emit_av(ha, o_ps, qh):
            for kc in range(TT):
                nc.tensor.matmul(
                    o_ps[:],
                    VA[kc][:, ha * (HD + 1) : (ha + 1) * (HD + 1)],
                    AT[(ha, kc)][:, qh * 512 : (qh + 1) * 512],
                    start=(kc == 0), stop=(kc == TT - 1),
                )

        state = {}  # ha -> (rb, o_scr)
        for i in range(H + 1):
            hs, ha = i + 1, i
            if ha < H:
                # both AV halves contiguous: all exps of head ha finished
                # last iteration, so the 16 matmuls run back-to-back with a
                # single stationary-kind transition (S<->AV switches cost
                # ~100ns each on the real PE)
                o_ps0 = ps_o.tile([HD + 1, 512], F32, tag="o", name=f"o{ha}_0")
                emit_av(ha, o_ps0, 0)
                o_ps1 = ps_o.tile([HD + 1, 512], F32, tag="o", name=f"o{ha}_1")
                emit_av(ha, o_ps1, 1)
            if hs < H:
                for kc in range(4):
                    emit_s_chunk(hs, kc)
            if ha < H:
                # custom DVE ops need SBUF input (PSUM reads silently corrupt
                # on HW) — copy the denominator rows out first
                dn_t = p_rdn.tile([1, NTOK], F32, tag="dn", name=f"dn{ha}")
                nc.vector.tensor_copy(dn_t[:, 0:512], o_ps0[HD : HD + 1, :])
                nc.vector.tensor_copy(dn_t[:, 512:1024], o_ps1[HD : HD + 1, :])
                r_t = p_rr.tile([1, NTOK], F32, tag="r", name=f"r{ha}")
                nc.vector.reciprocal_approx_fast(r_t[:], dn_t[:])
                rb = p_rb.tile([HD, NTOK], F32, tag="rb", name=f"rb{ha}")
                nc.gpsimd.partition_broadcast(rb[:], r_t[:], channels=HD)
            if hs < H:
                for kc in range(4, 8):
                    emit_s_chunk(hs, kc)
            if ha < H:
                o_scr = p_os.tile([HD, NTOK], BF16, tag="os", name=f"os{ha}")
                nc.vector.tensor_copy(o_scr[:, 0:512], o_ps0[0:HD, :])
                nc.vector.tensor_copy(o_scr[:, 512:1024], o_ps1[0:HD, :])
                state[ha] = (rb, o_scr)
            qc = i // 2 + 1
            if i % 2 == 0 and qc < DT:
                emit_q_chain(qc, evac_dve=True)
            hm = i - 1
            if 0 <= hm < H:
                rb_m, os_m = state.pop(hm)
                qt, po = hm // 2, (hm % 2) * HD
                nc.vector.tensor_mul(OT[qt][po : po + HD, :], os_m[:], rb_m[:])
                for kc in range(TT):
                    del AT[(hm, kc)]

        es3.close()  # ps_o
        psA.close()  # shared psum -> 8 banks free for ps_y
        es2.close()
        esK.close()

        # ---- phase 3: output projection (DVE bias-add evac) ----
        p_y = es4.enter_context(tc.tile_pool(name="p_y", bufs=3))
        ps_y = es4.enter_context(tc.tile_pool(name="ps_y", bufs=2, space="PSUM"))
        for qc in range(TT):
            y_ps = ps_y.tile([P, D], F32, tag="y", name=f"yps{qc}")
            for nh in range(2):
                sl = slice(nh * 512, (nh + 1) * 512)
                for dt in range(DT):
                    nc.tensor.matmul(
                        y_ps[:, sl], OT[dt][:, qc * P : (qc + 1) * P],
                        WO[dt][:, sl],
                        start=(dt == 0), stop=(dt == DT - 1),
                    )
            y_t = p_y.tile([P, D], F32, tag="yt", name=f"yt{qc}")
            nc.vector.tensor_add(y_t[:], y_ps[:], bob[:])
            nc.sync.dma_start(out_ext.ap()[qc * P : (qc + 1) * P, :], y_t[:])
        es4.close()
        esW.close()
        es0.close()

    nc.compile()
    return nc


# ---------------------------------------------------------------------------
# host side
# ---------------------------------------------------------------------------

def _pmajor(a):
    """[DT*P, N] -> [P, DT*N] partition-major bf16 (contiguous 16KB rows)."""
    d, n = a.shape
    return np.ascontiguousarray(
        a.reshape(DT, P, n).transpose(1, 0, 2).reshape(P, DT * n)
    ).astype(BF)


def host_prep(x, coords, is_context, rope_cache,
              ctx_in_w, ctx_in_b, ctx_out_w, ctx_out_b,
              tgt_in_w, tgt_in_b, tgt_out_w, tgt_out_b):
    x = np.asarray(x, np.float32)
    coords = np.asarray(coords, np.float32)
    is_context = np.asarray(is_context, bool)
    rope_cache = np.asarray(rope_cache, np.float32)

    keys = np.where(is_context, 0, 1).astype(np.int32)
    order = np.argsort(keys, axis=1, kind="stable")
    ctx_idx = order[:, :NCTX]
    tgt_idx = order[:, NCTX:]

    # rope rotation (mirrors reference fp32 arithmetic)
    cn = np.clip(
        coords / np.float32(IMAGE_SIZE) * np.float32(MAX_POS - 1), 0, MAX_POS - 1
    )
    y_pos = cn[..., 0].astype(np.int32)
    x_pos = cn[..., 1].astype(np.int32)
    cx = rope_cache[x_pos, :, 0]
    sx = rope_cache[x_pos, :, 1]
    cy = rope_cache[y_pos, :, 0]
    sy = rope_cache[y_pos, :, 1]
    half = D // 2
    xr = np.empty_like(x)
    xe = x[:, :, 0:half:2]
    xo = x[:, :, 1:half:2]
    xr[:, :, 0:half:2] = xe * cx - xo * sx
    xr[:, :, 1:half:2] = xe * sx + xo * cx
    ye = x[:, :, half::2]
    yo = x[:, :, half + 1 :: 2]
    xr[:, :, half::2] = ye * cy - yo * sy
    xr[:, :, half + 1 :: 2] = ye * sy + yo * cy

    def pack_w(in_w, in_b, out_w, out_b):
        w = np.array(in_w, np.float32)
        b3 = np.array(in_b, np.float32).copy()
        w[0:D] *= np.float32(0.125)  # fold 1/sqrt(hd) into Wq
        b3[0:D] *= np.float32(0.125)
        wT = np.ascontiguousarray(w.T)  # [D, 3D]
        wq = _pmajor(np.ascontiguousarray(wT[:, 0:D]))
        wk = _pmajor(np.ascontiguousarray(wT[:, D : 2 * D]))
        wv = _pmajor(np.ascontiguousarray(wT[:, 2 * D :]))
        wo = _pmajor(np.ascontiguousarray(np.asarray(out_w, np.float32).T))
        bvb = np.broadcast_to(b3[None, 2 * D :], (P, D)).astype(BF)
        bob = np.ascontiguousarray(
            np.broadcast_to(np.asarray(out_b, np.float32)[None, :], (P, D))
        )
        biasqk = np.zeros((P, 2 * DT), np.float32)
        biasqk[:, 0:DT] = b3[0:D].reshape(DT, P).T
        biasqk[:, DT:] = b3[D : 2 * D].reshape(DT, P).T
        return wq, wk, wv, wo, bvb, bob, biasqk

    packs = [pack_w(ctx_in_w, ctx_in_b, ctx_out_w, ctx_out_b),
             pack_w(tgt_in_w, tgt_in_b, tgt_out_w, tgt_out_b)]

    in_maps = []
    scatter = []
    for c in range(8):
        b, role = c // 2, c % 2
        q_idx = ctx_idx[b] if role == 0 else tgt_idx[b]
        kv_idx = ctx_idx[b]
        wq, wk, wv, wo, bvb, bob, biasqk = packs[role]
        in_maps.append({
            "xqT": _pmajor(np.ascontiguousarray(xr[b][q_idx].T)),
            "xkT": _pmajor(np.ascontiguousarray(xr[b][kv_idx].T)),
            "wq": wq, "wk": wk, "wv": wv, "wo": wo,
            "bvb": bvb, "bob": bob, "biasqk": biasqk,
        })
        scatter.append((b, q_idx))
    return in_maps, scatter


_NC_CACHE = None


def kernel(**inputs):
    global _NC_CACHE
    in_maps, scatter = host_prep(**inputs)
    if _NC_CACHE is None:
        _NC_CACHE = build_nc()
    nc = _NC_CACHE
    res = run_bass_kernel_spmd(nc, in_maps, core_ids=list(range(8)))
    x = np.asarray(inputs["x"], np.float32)
    out = np.zeros_like(x)
    for c in range(8):
        b, q_idx = scatter[c]
        out[b][q_idx] = res.results[c]["out"]
    return out


# revision 15
# speedup vs baseline: 1.0082x; 1.0082x over previous
"""Trainium2 Bass kernel for nn_CNNCrossPatchBackbone (sparse cross-patch attention).

Strategy: 8 cores = 4 batches x {ctx self-attention, tgt cross-attention}.
Fully task-parallel, no collectives. Each core: 1024 q-tokens x 1024
kv-tokens, 16 heads of dim 64, D=1024.

v3.5 vs v3.3 (308.7us):
  * All 1-partition bias matmuls removed: V bias and out-proj bias applied
    by the DVE during PSUM evacuation (tensor_add against host-prebroadcast
    bias tiles). Saves ~10us of PE at 317ns per bias matmul.
  * Q-projection chains 1..7 moved INTO the phase-2 head loop (one chain
    every other head), filling the PE slack under the ACT exp floor; Q bias
    applied via DVE tensor_scalar_add (per-partition scalar). Shrinks the
    serial phase-1 region by ~24us.
  * Phase 2 per head: 8 uniform 1024-wide exp chunks (shared 3-buf psum),
    AV into two single-bank [65,512] psums (full-speed 216ns AV matmuls;
    the 2-bank [65,1024] variant ran at 334ns), denominator reciprocal on
    SBUF copy (custom DVE ops corrupt on PSUM input), GPS broadcast +
    mixed-dtype normalize-mul lag one head behind, off the critical path.

Measured engine floors per core: ACT exp 8.5us/head (phase-2 pacer), PE
~216ns/512-col matmul when uncoupled. Predicted ~275us.
"""

import sys

sys.path.insert(0, "/opt/trn_rl_repo")

import ml_dtypes
import numpy as np

import concourse.bass as bass  # noqa: F401
import concourse.tile as tile
from concourse import bacc, mybir
from concourse.bass_utils import run_bass_kernel_spmd

B, K, D, H = 4, 2048, 1024, 16
NCTX = K // 2
NTOK = 1024
HD = D // H  # 64
IMAGE_SIZE = 224.0
MAX_POS = 1024
P = 128
DT = D // P  # 8
TT = NTOK // P  # 8
F32 = mybir.dt.float32
BF16 = mybir.dt.bfloat16
BF = ml_dtypes.bfloat16


def build_nc():
    nc = bacc.Bacc("TRN2", target_bir_lowering=False, debug=False, num_devices=8)

    xqT_ext = nc.dram_tensor("xqT", [P, DT * NTOK], BF16, kind="ExternalInput")
    xkT_ext = nc.dram_tensor("xkT", [P, DT * NTOK], BF16, kind="ExternalInput")
    wq_ext = nc.dram_tensor("wq", [P, DT * D], BF16, kind="ExternalInput")
    wk_ext = nc.dram_tensor("wk", [P, DT * D], BF16, kind="ExternalInput")
    wv_ext = nc.dram_tensor("wv", [P, DT * D], BF16, kind="ExternalInput")
    wo_ext = nc.dram_tensor("wo", [P, DT * D], BF16, kind="ExternalInput")
    bvb_ext = nc.dram_tensor("bvb", [P, D], BF16, kind="ExternalInput")
    bob_ext = nc.dram_tensor("bob", [P, D], F32, kind="ExternalInput")
    biasqk_ext = nc.dram_tensor("biasqk", [P, 2 * DT], F32, kind="ExternalInput")
    out_ext = nc.dram_tensor("out", [NTOK, D], F32, kind="ExternalOutput")

    from contextlib import ExitStack

    with tile.TileContext(nc) as tc:
        es0 = ExitStack()   # whole-kernel sbuf pools
        psA = ExitStack()   # shared [128,1024]f32 psum (proj chains + S chunks)
        esW = ExitStack()   # wo staging (lives to end)
        esK = ExitStack()   # wv/xk/wq/xq staging (live through phase 2)
        es1a = ExitStack()  # wk staging (closes after K chains)
        es2 = ExitStack()   # phase-2 sbuf pools (open after K chains)
        es3 = ExitStack()   # phase-2 AV psum
        es4 = ExitStack()   # phase-3 pools

        cpool = es0.enter_context(tc.tile_pool(name="const", bufs=1))
        p_qt = es0.enter_context(tc.tile_pool(name="p_qt", bufs=3))
        p_spr = es0.enter_context(tc.tile_pool(name="p_spr", bufs=DT))
        p_va = es0.enter_context(tc.tile_pool(name="p_va", bufs=TT))
        p_ot = es0.enter_context(tc.tile_pool(name="p_ot", bufs=DT))

        ones_f = cpool.tile([P, P], F32)
        nc.gpsimd.memset(ones_f[:], 1.0)
        ones_bf = cpool.tile([P, P], BF16)
        nc.vector.tensor_copy(ones_bf[:], ones_f[:])
        biasT = cpool.tile([P, 2 * DT], F32)
        nc.sync.dma_start(biasT[:], biasqk_ext.ap())
        bvb = cpool.tile([P, D], BF16)
        nc.scalar.dma_start(bvb[:], bvb_ext.ap())
        bob = cpool.tile([P, D], F32)
        nc.scalar.dma_start(bob[:], bob_ext.ap())

        QT = {}  # chain idx -> [128, NTOK] bf16 tile
        # SPRP[c] holds K^T for head 2c in rows 0:64 and head 2c+1 in rows 64:128
        SPRP = [p_spr.tile([P, NTOK], BF16, tag="spr", name=f"spr{c}") for c in range(DT)]
        VA = [p_va.tile([P, H * (HD + 1)], BF16, tag="va", name=f"va{i}") for i in range(TT)]
        OT = [p_ot.tile([P, NTOK], BF16, tag="ot", name=f"ot{i}") for i in range(DT)]
        for va in VA:
            nc.vector.tensor_copy(
                va[:].rearrange("p (h c) -> p h c", c=HD + 1)[:, :, HD : HD + 1],
                ones_bf[:, 0:H].rearrange("p (h c) -> p h c", c=1),
            )

        # ---- staging + DMAs ----
        ps = psA.enter_context(tc.tile_pool(name="ps", bufs=3, space="PSUM"))
        p_wo = esW.enter_context(tc.tile_pool(name="p_wo", bufs=1))
        p_wv = esK.enter_context(tc.tile_pool(name="p_wv", bufs=1))
        p_xk = esK.enter_context(tc.tile_pool(name="p_xk", bufs=1))
        p_wq = esK.enter_context(tc.tile_pool(name="p_wq", bufs=1))
        p_xq = esK.enter_context(tc.tile_pool(name="p_xq", bufs=1))
        p_wk = es1a.enter_context(tc.tile_pool(name="p_wk", bufs=1))

        def stage(pool, n, tag):
            t = pool.tile([P, n * NTOK], BF16, tag=tag, name=tag)
            return t, [t[:, i * NTOK : (i + 1) * NTOK] for i in range(n)]

        # wk/xk split per-dt on two queues: K chains start as tiles land.
        wk_t, WK = stage(p_wk, DT, "wk")
        xk_t, XK = stage(p_xk, DT, "xk")
        for dt in range(DT):
            nc.sync.dma_start(
                wk_t[:, dt * NTOK : (dt + 1) * NTOK],
                wk_ext.ap()[:, dt * NTOK : (dt + 1) * NTOK],
            )
            nc.gpsimd.dma_start(
                xk_t[:, dt * NTOK : (dt + 1) * NTOK],
                xkT_ext.ap()[:, dt * NTOK : (dt + 1) * NTOK],
            )
        wq_t, WQ = stage(p_wq, DT, "wq")
        nc.sync.dma_start(wq_t[:], wq_ext.ap())
        xq_t, XQ = stage(p_xq, DT, "xq")
        nc.gpsimd.dma_start(xq_t[:], xqT_ext.ap())
        wv_t, WV = stage(p_wv, DT, "wv")
        nc.sync.dma_start(wv_t[:], wv_ext.ap())
        wo_t, WO = stage(p_wo, DT, "wo")
        nc.sync.dma_start(wo_t[:], wo_ext.ap())

        # ---- phase 1: K chains + Q chain 0 (ACT identity evac) ----
        for c in range(DT):
            kps = ps.tile([P, NTOK], F32, tag="ps", name=f"kps{c}")
            for nh in range(2):
                sl = slice(nh * 512, (nh + 1) * 512)
                for dt in range(DT):
                    nc.tensor.matmul(
                        kps[:, sl], WK[dt][:, c * P : (c + 1) * P], XK[dt][:, sl],
                        start=(dt == 0), stop=(dt == DT - 1),
                    )
            nc.scalar.activation(
                SPRP[c][0:HD, :], kps[0:HD, :],
                mybir.ActivationFunctionType.Identity,
                bias=biasT[0:HD, DT + c : DT + c + 1],
            )
            nc.scalar.activation(
                SPRP[c][HD:P, :], kps[HD:P, :],
                mybir.ActivationFunctionType.Identity,
                bias=biasT[HD:P, DT + c : DT + c + 1],
            )

        def emit_q_chain(c, evac_dve):
            qps = ps.tile([P, NTOK], F32, tag="ps", name=f"qps{c}")
            for nh in range(2):
                sl = slice(nh * 512, (nh + 1) * 512)
                for dt in range(DT):
                    nc.tensor.matmul(
                        qps[:, sl], WQ[dt][:, c * P : (c + 1) * P], XQ[dt][:, sl],
                        start=(dt == 0), stop=(dt == DT - 1),
                    )
            QT[c] = p_qt.tile([P, NTOK], BF16, tag="qt", name=f"qt{c}")
            if evac_dve:
                nc.vector.tensor_scalar_add(QT[c][:], qps[:], biasT[:, c : c + 1])
            else:
                nc.scalar.activation(
                    QT[c][:], qps[:],
                    mybir.ActivationFunctionType.Identity,
                    bias=biasT[:, c : c + 1],
                )

        emit_q_chain(0, evac_dve=False)
        es1a.close()  # wk staging done

        # ---- phase-2 sbuf pools ----
        p_a = es2.enter_context(tc.tile_pool(name="p_a", bufs=14))
        p_rdn = es2.enter_context(tc.tile_pool(name="p_rdn", bufs=1))
        p_rr = es2.enter_context(tc.tile_pool(name="p_rr", bufs=1))
        p_rb = es2.enter_context(tc.tile_pool(name="p_rb", bufs=2))
        p_os = es2.enter_context(tc.tile_pool(name="p_os", bufs=2))

        AT = {}  # (h, kc) -> a tile [128k, 1024q] bf16

        def emit_s_chunk(h, kc):
            qt, po = h // 2, (h % 2) * HD
            s_ps = ps.tile([P, NTOK], F32, tag="ps", name=f"s{h}_{kc}")
            for j in range(2):
                nc.tensor.matmul(
                    s_ps[:, j * 512 : (j + 1) * 512],
                    SPRP[h // 2][po : po + HD, kc * P : (kc + 1) * P],
                    QT[qt][po : po + HD, j * 512 : (j + 1) * 512],
                    start=True, stop=True,
                )
            a_t = p_a.tile([P, NTOK], BF16, tag="a", name=f"a{h}_{kc}")
            nc.scalar.activation(a_t[:], s_ps[:], mybir.ActivationFunctionType.Exp)
            AT[(h, kc)] = a_t

        # V chains (DVE bias-add evac) interleaved with S/exp of head 0
        for tt in range(TT):
            vps = ps.tile([P, NTOK], F32, tag="ps", name=f"vps{tt}")
            for nh in range(2):
                sl = slice(nh * 512, (nh + 1) * 512)
                for dt in range(DT):
                    nc.tensor.matmul(
                        vps[:, sl], XK[dt][:, tt * P : (tt + 1) * P],
                        WV[dt][:, sl],
                        start=(dt == 0), stop=(dt == DT - 1),
                    )
            nc.vector.tensor_add(
                VA[tt][:].rearrange("p (h c) -> p h c", c=HD + 1)[:, :, 0:HD],
                bvb[:].rearrange("p (h c) -> p h c", c=HD),
                vps[:].rearrange("p (h c) -> p h c", c=HD),
            )
            emit_s_chunk(0, tt)

        ps_o = es3.enter_context(tc.tile_pool(name="ps_o", bufs=2, space="PSUM"))

        # ---- phase 2 main loop: S(i+1) | AV(i) | Q-chain | norm(i) | mul(i-1)
        def emit_av(ha, o_ps, qh):
            for kc in range(TT):
                nc.tensor.matmul(
                    o_ps[:],
                    VA[kc][:, ha * (HD + 1) : (ha + 1) * (HD + 1)],
                    AT[(ha, kc)][:, qh * 512 : (qh + 1) * 512],
                    start=(kc == 0), stop=(kc == TT - 1),
                )

        state = {}  # ha -> (rb, o_scr)
        for i in range(H + 1):
            hs, ha = i + 1, i
            if hs < H:
                # two S chunks first so the ACT exp stream never starves
                emit_s_chunk(hs, 0)
                emit_s_chunk(hs, 1)
            if ha < H:
                # both AV halves contiguous: all exps of head ha finished by
                # early this iteration, and the previous head's PSUM was freed
                # by mid-iteration evacs — so the 16 matmuls run back-to-back
                # (S<->AV stationary-kind switches cost ~100ns on the PE)
                o_ps0 = ps_o.tile([HD + 1, 512], F32, tag="o", name=f"o{ha}_0")
                emit_av(ha, o_ps0, 0)
                o_ps1 = ps_o.tile([HD + 1, 512], F32, tag="o", name=f"o{ha}_1")
                emit_av(ha, o_ps1, 1)
            if hs < H:
                emit_s_chunk(hs, 2)
                emit_s_chunk(hs, 3)
            if ha < H:
                # custom DVE ops need SBUF input (PSUM reads silently corrupt
                # on HW) — copy the denominator rows out first. Evacuate the
                # unnormalized o immediately after: frees both AV psum banks
                # well before the next head's AV block.
                dn_t = p_rdn.tile([1, NTOK], F32, tag="dn", name=f"dn{ha}")
                nc.vector.tensor_copy(dn_t[:, 0:512], o_ps0[HD : HD + 1, :])
                nc.vector.tensor_copy(dn_t[:, 512:1024], o_ps1[HD : HD + 1, :])
                r_t = p_rr.tile([1, NTOK], F32, tag="r", name=f"r{ha}")
                nc.vector.reciprocal_approx_fast(r_t[:], dn_t[:])
                rb = p_rb.tile([HD, NTOK], F32, tag="rb", name=f"rb{ha}")
                nc.gpsimd.partition_broadcast(rb[:], r_t[:], channels=HD)
                o_scr = p_os.tile([HD, NTOK], BF16, tag="os", name=f"os{ha}")
                nc.vector.tensor_copy(o_scr[:, 0:512], o_ps0[0:HD, :])
                nc.vector.tensor_copy(o_scr[:, 512:1024], o_ps1[0:HD, :])
                state[ha] = (rb, o_scr)
            if hs < H:
                for kc in range(4, 8):
                    emit_s_chunk(hs, kc)
            qc = i // 2 + 1
            if i % 2 == 0 and qc < DT:
                emit_q_chain(qc, evac_dve=True)
            hm = i - 1
            if 0 <= hm < H:
                rb_m, os_m = state.pop(hm)
                qt, po = hm // 2, (hm % 2) * HD
                nc.vector.tensor_mul(OT[qt][po : po + HD, :], os_m[:], rb_m[:])
                for kc in range(TT):
                    del AT[(hm, kc)]

        es3.close()  # ps_o
        psA.close()  # shared psum -> 8 banks free for ps_y
        es2.close()
        esK.close()

        # ---- phase 3: output projection (DVE bias-add evac) ----
        p_y = es4.enter_context(tc.tile_pool(name="p_y", bufs=3))
        ps_y = es4.enter_context(tc.tile_pool(name="ps_y", bufs=2, space="PSUM"))
        for qc in range(TT):
            y_ps = ps_y.tile([P, D], F32, tag="y", name=f"yps{qc}")
            for nh in range(2):
                sl = slice(nh * 512, (nh + 1) * 512)
                for dt in range(DT):
                    nc.tensor.matmul(
                        y_ps[:, sl], OT[dt][:, qc * P : (qc + 1) * P],
                        WO[dt][:, sl],
                        start=(dt == 0), stop=(dt == DT - 1),
                    )
            y_t = p_y.tile([P, D], F32, tag="yt", name=f"yt{qc}")
            nc.vector.tensor_add(y_t[:], y_ps[:], bob[:])
            nc.sync.dma_start(out_ext.ap()[qc * P : (qc + 1) * P, :], y_t[:])
        es4.close()
        esW.close()
        es0.close()

    nc.compile()
    return nc


# ---------------------------------------------------------------------------
# host side
# ---------------------------------------------------------------------------

def _pmajor(a):
    """[DT*P, N] -> [P, DT*N] partition-major bf16 (contiguous 16KB rows)."""
    d, n = a.shape
    return np.ascontiguousarray(
        a.reshape(DT, P, n).transpose(1, 0, 2).reshape(P, DT * n)
    ).astype(BF)


def host_prep(x, coords, is_context, rope_cache,
              ctx_in_w, ctx_in_b, ctx_out_w, ctx_out_b,
              tgt_in_w, tgt_in_b, tgt_out_w, tgt_out_b):
    x = np.asarray(x, np.float32)
    coords = np.asarray(coords, np.float32)
    is_context = np.asarray(is_context, bool)
    rope_cache = np.asarray(rope_cache, np.float32)

    keys = np.where(is_context, 0, 1).astype(np.int32)
    order = np.argsort(keys, axis=1, kind="stable")
    ctx_idx = order[:, :NCTX]
    tgt_idx = order[:, NCTX:]

    # rope rotation (mirrors reference fp32 arithmetic)
    cn = np.clip(
        coords / np.float32(IMAGE_SIZE) * np.float32(MAX_POS - 1), 0, MAX_POS - 1
    )
    y_pos = cn[..., 0].astype(np.int32)
    x_pos = cn[..., 1].astype(np.int32)
    cx = rope_cache[x_pos, :, 0]
    sx = rope_cache[x_pos, :, 1]
    cy = rope_cache[y_pos, :, 0]
    sy = rope_cache[y_pos, :, 1]
    half = D // 2
    xr = np.empty_like(x)
    xe = x[:, :, 0:half:2]
    xo = x[:, :, 1:half:2]
    xr[:, :, 0:half:2] = xe * cx - xo * sx
    xr[:, :, 1:half:2] = xe * sx + xo * cx
    ye = x[:, :, half::2]
    yo = x[:, :, half + 1 :: 2]
    xr[:, :, half::2] = ye * cy - yo * sy
    xr[:, :, half + 1 :: 2] = ye * sy + yo * cy

    def pack_w(in_w, in_b, out_w, out_b):
        w = np.array(in_w, np.float32)
        b3 = np.array(in_b, np.float32).copy()
        w[0:D] *= np.float32(0.125)  # fold 1/sqrt(hd) into Wq
        b3[0:D] *= np.float32(0.125)
        wT = np.ascontiguousarray(w.T)  # [D, 3D]
        wq = _pmajor(np.ascontiguousarray(wT[:, 0:D]))
        wk = _pmajor(np.ascontiguousarray(wT[:, D : 2 * D]))
        wv = _pmajor(np.ascontiguousarray(wT[:, 2 * D :]))
        wo = _pmajor(np.ascontiguousarray(np.asarray(out_w, np.float32).T))
        bvb = np.broadcast_to(b3[None, 2 * D :], (P, D)).astype(BF)
        bob = np.ascontiguousarray(
            np.broadcast_to(np.asarray(out_b, np.float32)[None, :], (P, D))
        )
        biasqk = np.zeros((P, 2 * DT), np.float32)
        biasqk[:, 0:DT] = b3[0:D].reshape(DT, P).T
        biasqk[:, DT:] = b3[D : 2 * D].reshape(DT, P).T
        return wq, wk, wv, wo, bvb, bob, biasqk

    packs = [pack_w(ctx_in_w, ctx_in_b, ctx_out_w, ctx_out_b),
             pack_w(tgt_in_w, tgt_in_b, tgt_out_w, tgt_out_b)]

    in_maps = []
    scatter = []
    for c in range(8):
        b, role = c // 2, c % 2
        q_idx = ctx_idx[b] if role == 0 else tgt_idx[b]
        kv_idx = ctx_idx[b]
        wq, wk, wv, wo, bvb, bob, biasqk = packs[role]
        in_maps.append({
            "xqT": _pmajor(np.ascontiguousarray(xr[b][q_idx].T)),
            "xkT": _pmajor(np.ascontiguousarray(xr[b][kv_idx].T)),
            "wq": wq, "wk": wk, "wv": wv, "wo": wo,
            "bvb": bvb, "bob": bob, "biasqk": biasqk,
        })
        scatter.append((b, q_idx))
    return in_maps, scatter


_NC_CACHE = None


def kernel(**inputs):
    global _NC_CACHE
    in_maps, scatter = host_prep(**inputs)
    if _NC_CACHE is None:
        _NC_CACHE = build_nc()
    nc = _NC_CACHE
    res = run_bass_kernel_spmd(nc, in_maps, core_ids=list(range(8)))
    x = np.asarray(inputs["x"], np.float32)
    out = np.zeros_like(x)
    for c in range(8):
        b, q_idx = scatter[c]
        out[b][q_idx] = res.results[c]["out"]
    return out


# revision 17
# speedup vs baseline: 1.0467x; 1.0382x over previous
"""Trainium2 Bass kernel for nn_CNNCrossPatchBackbone (sparse cross-patch attention).

Strategy: 8 cores = 4 batches x {ctx self-attention, tgt cross-attention}.
Fully task-parallel, no collectives. Each core: 1024 q-tokens x 1024
kv-tokens, 16 heads of dim 64, D=1024.

v3.5 vs v3.3 (308.7us):
  * All 1-partition bias matmuls removed: V bias and out-proj bias applied
    by the DVE during PSUM evacuation (tensor_add against host-prebroadcast
    bias tiles). Saves ~10us of PE at 317ns per bias matmul.
  * Q-projection chains 1..7 moved INTO the phase-2 head loop (one chain
    every other head), filling the PE slack under the ACT exp floor; Q bias
    applied via DVE tensor_scalar_add (per-partition scalar). Shrinks the
    serial phase-1 region by ~24us.
  * Phase 2 per head: 8 uniform 1024-wide exp chunks (shared 3-buf psum),
    AV into two single-bank [65,512] psums (full-speed 216ns AV matmuls;
    the 2-bank [65,1024] variant ran at 334ns), denominator reciprocal on
    SBUF copy (custom DVE ops corrupt on PSUM input), GPS broadcast +
    mixed-dtype normalize-mul lag one head behind, off the critical path.

Measured engine floors per core: ACT exp 8.5us/head (phase-2 pacer), PE
~216ns/512-col matmul when uncoupled. Predicted ~275us.
"""

import sys

sys.path.insert(0, "/opt/trn_rl_repo")

import ml_dtypes
import numpy as np

import concourse.bass as bass  # noqa: F401
import concourse.tile as tile
from concourse import bacc, mybir
from concourse.bass_utils import run_bass_kernel_spmd

B, K, D, H = 4, 2048, 1024, 16
NCTX = K // 2
NTOK = 1024
HD = D // H  # 64
IMAGE_SIZE = 224.0
MAX_POS = 1024
P = 128
DT = D // P  # 8
TT = NTOK // P  # 8
F32 = mybir.dt.float32
BF16 = mybir.dt.bfloat16
BF = ml_dtypes.bfloat16


def build_nc():
    nc = bacc.Bacc("TRN2", target_bir_lowering=False, debug=False, num_devices=8)

    xqT_ext = nc.dram_tensor("xqT", [P, DT * NTOK], BF16, kind="ExternalInput")
    xkT_ext = nc.dram_tensor("xkT", [P, DT * NTOK], BF16, kind="ExternalInput")
    wq_ext = nc.dram_tensor("wq", [P, DT * D], BF16, kind="ExternalInput")
    wk_ext = nc.dram_tensor("wk", [P, DT * D], BF16, kind="ExternalInput")
    wv_ext = nc.dram_tensor("wv", [P, DT * D], BF16, kind="ExternalInput")
    wo_ext = nc.dram_tensor("wo", [P, DT * D], BF16, kind="ExternalInput")
    bvb_ext = nc.dram_tensor("bvb", [P, D], BF16, kind="ExternalInput")
    bob_ext = nc.dram_tensor("bob", [P, D], F32, kind="ExternalInput")
    biasqk_ext = nc.dram_tensor("biasqk", [P, 2 * DT], F32, kind="ExternalInput")
    out_ext = nc.dram_tensor("out", [NTOK, D], F32, kind="ExternalOutput")

    from contextlib import ExitStack

    with tile.TileContext(nc) as tc:
        es0 = ExitStack()   # whole-kernel sbuf pools
        psA = ExitStack()   # shared [128,1024]f32 psum (proj chains + S chunks)
        esW = ExitStack()   # wo staging (lives to end)
        esK = ExitStack()   # wv/xk/wq/xq staging (live through phase 2)
        es1a = ExitStack()  # wk staging (closes after K chains)
        es2 = ExitStack()   # phase-2 sbuf pools (open after K chains)
        es3 = ExitStack()   # phase-2 AV psum
        es4 = ExitStack()   # phase-3 pools

        cpool = es0.enter_context(tc.tile_pool(name="const", bufs=1))
        p_qt = es0.enter_context(tc.tile_pool(name="p_qt", bufs=3))
        p_spr = es0.enter_context(tc.tile_pool(name="p_spr", bufs=H))
        p_va = es0.enter_context(tc.tile_pool(name="p_va", bufs=TT))
        p_ot = es0.enter_context(tc.tile_pool(name="p_ot", bufs=DT))

        ones_f = cpool.tile([P, P], F32)
        nc.gpsimd.memset(ones_f[:], 1.0)
        ones_bf = cpool.tile([P, P], BF16)
        nc.vector.tensor_copy(ones_bf[:], ones_f[:])
        biasT = cpool.tile([P, 2 * DT], F32)
        nc.sync.dma_start(biasT[:], biasqk_ext.ap())
        bvb = cpool.tile([P, D], BF16)
        nc.scalar.dma_start(bvb[:], bvb_ext.ap())
        bob = cpool.tile([P, D], F32)
        nc.scalar.dma_start(bob[:], bob_ext.ap())

        QT = {}  # chain idx -> [128, NTOK] bf16 tile
        # SPR[h]: K^T for head h in rows (h%2)*64..+64, other rows ZERO so the
        # S matmul can contract all 128 partitions (uniform 128-part
        # stationaries keep PE kind-transitions cheap: 64<->128 partition
        # switches cost ~350ns/matmul, measured)
        SPR = [p_spr.tile([P, NTOK], BF16, tag="spr", name=f"spr{h}") for h in range(H)]
        VA = [p_va.tile([P, H * (HD + 1)], BF16, tag="va", name=f"va{i}") for i in range(TT)]
        OT = [p_ot.tile([P, NTOK], BF16, tag="ot", name=f"ot{i}") for i in range(DT)]
        for va in VA:
            nc.vector.tensor_copy(
                va[:].rearrange("p (h c) -> p h c", c=HD + 1)[:, :, HD : HD + 1],
                ones_bf[:, 0:H].rearrange("p (h c) -> p h c", c=1),
            )

        # ---- staging + DMAs ----
        ps = psA.enter_context(tc.tile_pool(name="ps", bufs=3, space="PSUM"))
        p_wo = esW.enter_context(tc.tile_pool(name="p_wo", bufs=1))
        p_wv = esK.enter_context(tc.tile_pool(name="p_wv", bufs=1))
        p_xk = esK.enter_context(tc.tile_pool(name="p_xk", bufs=1))
        p_wq = esK.enter_context(tc.tile_pool(name="p_wq", bufs=1))
        p_xq = esK.enter_context(tc.tile_pool(name="p_xq", bufs=1))
        p_wk = es1a.enter_context(tc.tile_pool(name="p_wk", bufs=1))

        def stage(pool, n, tag):
            t = pool.tile([P, n * NTOK], BF16, tag=tag, name=tag)
            return t, [t[:, i * NTOK : (i + 1) * NTOK] for i in range(n)]

        # wk/xk split per-dt on two queues: K chains start as tiles land.
        wk_t, WK = stage(p_wk, DT, "wk")
        xk_t, XK = stage(p_xk, DT, "xk")
        for dt in range(DT):
            nc.sync.dma_start(
                wk_t[:, dt * NTOK : (dt + 1) * NTOK],
                wk_ext.ap()[:, dt * NTOK : (dt + 1) * NTOK],
            )
            nc.gpsimd.dma_start(
                xk_t[:, dt * NTOK : (dt + 1) * NTOK],
                xkT_ext.ap()[:, dt * NTOK : (dt + 1) * NTOK],
            )
        wq_t, WQ = stage(p_wq, DT, "wq")
        nc.sync.dma_start(wq_t[:], wq_ext.ap())
        xq_t, XQ = stage(p_xq, DT, "xq")
        nc.gpsimd.dma_start(xq_t[:], xqT_ext.ap())
        wv_t, WV = stage(p_wv, DT, "wv")
        nc.sync.dma_start(wv_t[:], wv_ext.ap())
        wo_t, WO = stage(p_wo, DT, "wo")
        nc.sync.dma_start(wo_t[:], wo_ext.ap())
        # zero the unused half of each SPR tile (GPS is idle during the DMAs)
        for h in range(H):
            po = (h % 2) * HD
            nc.gpsimd.memset(SPR[h][HD - po : P - po, :], 0.0)

        # ---- phase 1: K chains + Q chain 0 (ACT identity evac) ----
        for c in range(DT):
            kps = ps.tile([P, NTOK], F32, tag="ps", name=f"kps{c}")
            for nh in range(2):
                sl = slice(nh * 512, (nh + 1) * 512)
                for dt in range(DT):
                    nc.tensor.matmul(
                        kps[:, sl], WK[dt][:, c * P : (c + 1) * P], XK[dt][:, sl],
                        start=(dt == 0), stop=(dt == DT - 1),
                    )
            nc.scalar.activation(
                SPR[2 * c][0:HD, :], kps[0:HD, :],
                mybir.ActivationFunctionType.Identity,
                bias=biasT[0:HD, DT + c : DT + c + 1],
            )
            nc.scalar.activation(
                SPR[2 * c + 1][HD:P, :], kps[HD:P, :],
                mybir.ActivationFunctionType.Identity,
                bias=biasT[HD:P, DT + c : DT + c + 1],
            )

        def emit_q_chain(c, evac_dve):
            qps = ps.tile([P, NTOK], F32, tag="ps", name=f"qps{c}")
            for nh in range(2):
                sl = slice(nh * 512, (nh + 1) * 512)
                for dt in range(DT):
                    nc.tensor.matmul(
                        qps[:, sl], WQ[dt][:, c * P : (c + 1) * P], XQ[dt][:, sl],
                        start=(dt == 0), stop=(dt == DT - 1),
                    )
            QT[c] = p_qt.tile([P, NTOK], BF16, tag="qt", name=f"qt{c}")
            if evac_dve:
                nc.vector.tensor_scalar_add(QT[c][:], qps[:], biasT[:, c : c + 1])
            else:
                nc.scalar.activation(
                    QT[c][:], qps[:],
                    mybir.ActivationFunctionType.Identity,
                    bias=biasT[:, c : c + 1],
                )

        emit_q_chain(0, evac_dve=False)
        es1a.close()  # wk staging done

        # ---- phase-2 sbuf pools ----
        p_a = es2.enter_context(tc.tile_pool(name="p_a", bufs=14))
        p_rdn = es2.enter_context(tc.tile_pool(name="p_rdn", bufs=1))
        p_rr = es2.enter_context(tc.tile_pool(name="p_rr", bufs=1))
        p_rb = es2.enter_context(tc.tile_pool(name="p_rb", bufs=2))
        p_os = es2.enter_context(tc.tile_pool(name="p_os", bufs=2))

        AT = {}  # (h, kc) -> a tile [128k, 1024q] bf16

        def emit_s_chunk(h, kc):
            qt, po = h // 2, (h % 2) * HD
            s_ps = ps.tile([P, NTOK], F32, tag="ps", name=f"s{h}_{kc}")
            for j in range(2):
                nc.tensor.matmul(
                    s_ps[:, j * 512 : (j + 1) * 512],
                    SPR[h][:, kc * P : (kc + 1) * P],
                    QT[qt][:, j * 512 : (j + 1) * 512],
                    start=True, stop=True,
                )
            a_t = p_a.tile([P, NTOK], BF16, tag="a", name=f"a{h}_{kc}")
            nc.scalar.activation(a_t[:], s_ps[:], mybir.ActivationFunctionType.Exp)
            AT[(h, kc)] = a_t

        # V chains (DVE bias-add evac) interleaved with S/exp of head 0
        for tt in range(TT):
            vps = ps.tile([P, NTOK], F32, tag="ps", name=f"vps{tt}")
            for nh in range(2):
                sl = slice(nh * 512, (nh + 1) * 512)
                for dt in range(DT):
                    nc.tensor.matmul(
                        vps[:, sl], XK[dt][:, tt * P : (tt + 1) * P],
                        WV[dt][:, sl],
                        start=(dt == 0), stop=(dt == DT - 1),
                    )
            nc.vector.tensor_add(
                VA[tt][:].rearrange("p (h c) -> p h c", c=HD + 1)[:, :, 0:HD],
                bvb[:].rearrange("p (h c) -> p h c", c=HD),
                vps[:].rearrange("p (h c) -> p h c", c=HD),
            )
            emit_s_chunk(0, tt)

        ps_o = es3.enter_context(tc.tile_pool(name="ps_o", bufs=2, space="PSUM"))

        # ---- phase 2 main loop: S(i+1) | AV(i) | Q-chain | norm(i) | mul(i-1)
        def emit_av(ha, o_ps, qh):
            for kc in range(TT):
                nc.tensor.matmul(
                    o_ps[:],
                    VA[kc][:, ha * (HD + 1) : (ha + 1) * (HD + 1)],
                    AT[(ha, kc)][:, qh * 512 : (qh + 1) * 512],
                    start=(kc == 0), stop=(kc == TT - 1),
                )

        state = {}  # ha -> (rb, o_scr)
        for i in range(H + 1):
            hs, ha = i + 1, i
            if hs < H:
                emit_s_chunk(hs, 0)
                emit_s_chunk(hs, 1)
            if ha < H:
                o_ps0 = ps_o.tile([HD + 1, 512], F32, tag="o", name=f"o{ha}_0")
                emit_av(ha, o_ps0, 0)
            if hs < H:
                emit_s_chunk(hs, 2)
                emit_s_chunk(hs, 3)
            if ha < H:
                o_ps1 = ps_o.tile([HD + 1, 512], F32, tag="o", name=f"o{ha}_1")
                emit_av(ha, o_ps1, 1)
            if hs < H:
                emit_s_chunk(hs, 4)
                emit_s_chunk(hs, 5)
            if ha < H:
                # custom DVE ops need SBUF input (PSUM reads silently corrupt
                # on HW) — copy the denominator rows out first
                dn_t = p_rdn.tile([1, NTOK], F32, tag="dn", name=f"dn{ha}")
                nc.vector.tensor_copy(dn_t[:, 0:512], o_ps0[HD : HD + 1, :])
                nc.vector.tensor_copy(dn_t[:, 512:1024], o_ps1[HD : HD + 1, :])
                r_t = p_rr.tile([1, NTOK], F32, tag="r", name=f"r{ha}")
                nc.vector.reciprocal_approx_fast(r_t[:], dn_t[:])
                rb = p_rb.tile([HD, NTOK], F32, tag="rb", name=f"rb{ha}")
                nc.gpsimd.partition_broadcast(rb[:], r_t[:], channels=HD)
            qc = i // 2 + 1
            if i % 2 == 0 and qc < DT:
                emit_q_chain(qc, evac_dve=True)
            if hs < H:
                emit_s_chunk(hs, 6)
                emit_s_chunk(hs, 7)
            if ha < H:
                o_scr = p_os.tile([HD, NTOK], BF16, tag="os", name=f"os{ha}")
                nc.vector.tensor_copy(o_scr[:, 0:512], o_ps0[0:HD, :])
                nc.vector.tensor_copy(o_scr[:, 512:1024], o_ps1[0:HD, :])
                state[ha] = (rb, o_scr)
            hm = i - 1
            if 0 <= hm < H:
                rb_m, os_m = state.pop(hm)
                qt, po = hm // 2, (hm % 2) * HD
                nc.vector.tensor_mul(OT[qt][po : po + HD, :], os_m[:], rb_m[:])
                for kc in range(TT):
                    del AT[(hm, kc)]

        es3.close()  # ps_o
        psA.close()  # shared psum -> 8 banks free for ps_y
        es2.close()
        esK.close()

        # ---- phase 3: output projection (DVE bias-add evac) ----
        p_y = es4.enter_context(tc.tile_pool(name="p_y", bufs=3))
        ps_y = es4.enter_context(tc.tile_pool(name="ps_y", bufs=2, space="PSUM"))
        for qc in range(TT):
            y_ps = ps_y.tile([P, D], F32, tag="y", name=f"yps{qc}")
            for nh in range(2):
                sl = slice(nh * 512, (nh + 1) * 512)
                for dt in range(DT):
                    nc.tensor.matmul(
                        y_ps[:, sl], OT[dt][:, qc * P : (qc + 1) * P],
                        WO[dt][:, sl],
                        start=(dt == 0), stop=(dt == DT - 1),
                    )
            y_t = p_y.tile([P, D], F32, tag="yt", name=f"yt{qc}")
            nc.vector.tensor_add(y_t[:], y_ps[:], bob[:])
            nc.sync.dma_start(out_ext.ap()[qc * P : (qc + 1) * P, :], y_t[:])
        es4.close()
        esW.close()
        es0.close()

    nc.compile()
    return nc


# ---------------------------------------------------------------------------
# host side
# ---------------------------------------------------------------------------

def _pmajor(a):
    """[DT*P, N] -> [P, DT*N] partition-major bf16 (contiguous 16KB rows)."""
    d, n = a.shape
    return np.ascontiguousarray(
        a.reshape(DT, P, n).transpose(1, 0, 2).reshape(P, DT * n)
    ).astype(BF)


def host_prep(x, coords, is_context, rope_cache,
              ctx_in_w, ctx_in_b, ctx_out_w, ctx_out_b,
              tgt_in_w, tgt_in_b, tgt_out_w, tgt_out_b):
    x = np.asarray(x, np.float32)
    coords = np.asarray(coords, np.float32)
    is_context = np.asarray(is_context, bool)
    rope_cache = np.asarray(rope_cache, np.float32)

    keys = np.where(is_context, 0, 1).astype(np.int32)
    order = np.argsort(keys, axis=1, kind="stable")
    ctx_idx = order[:, :NCTX]
    tgt_idx = order[:, NCTX:]

    # rope rotation (mirrors reference fp32 arithmetic)
    cn = np.clip(
        coords / np.float32(IMAGE_SIZE) * np.float32(MAX_POS - 1), 0, MAX_POS - 1
    )
    y_pos = cn[..., 0].astype(np.int32)
    x_pos = cn[..., 1].astype(np.int32)
    cx = rope_cache[x_pos, :, 0]
    sx = rope_cache[x_pos, :, 1]
    cy = rope_cache[y_pos, :, 0]
    sy = rope_cache[y_pos, :, 1]
    half = D // 2
    xr = np.empty_like(x)
    xe = x[:, :, 0:half:2]
    xo = x[:, :, 1:half:2]
    xr[:, :, 0:half:2] = xe * cx - xo * sx
    xr[:, :, 1:half:2] = xe * sx + xo * cx
    ye = x[:, :, half::2]
    yo = x[:, :, half + 1 :: 2]
    xr[:, :, half::2] = ye * cy - yo * sy
    xr[:, :, half + 1 :: 2] = ye * sy + yo * cy

    def pack_w(in_w, in_b, out_w, out_b):
        w = np.array(in_w, np.float32)
        b3 = np.array(in_b, np.float32).copy()
        w[0:D] *= np.float32(0.125)  # fold 1/sqrt(hd) into Wq
        b3[0:D] *= np.float32(0.125)
        wT = np.ascontiguousarray(w.T)  # [D, 3D]
        wq = _pmajor(np.ascontiguousarray(wT[:, 0:D]))
        wk = _pmajor(np.ascontiguousarray(wT[:, D : 2 * D]))
        wv = _pmajor(np.ascontiguousarray(wT[:, 2 * D :]))
        wo = _pmajor(np.ascontiguousarray(np.asarray(out_w, np.float32).T))
        bvb = np.broadcast_to(b3[None, 2 * D :], (P, D)).astype(BF)
        bob = np.ascontiguousarray(
            np.broadcast_to(np.asarray(out_b, np.float32)[None, :], (P, D))
        )
        biasqk = np.zeros((P, 2 * DT), np.float32)
        biasqk[:, 0:DT] = b3[0:D].reshape(DT, P).T
        biasqk[:, DT:] = b3[D : 2 * D].reshape(DT, P).T
        return wq, wk, wv, wo, bvb, bob, biasqk

    packs = [pack_w(ctx_in_w, ctx_in_b, ctx_out_w, ctx_out_b),
             pack_w(tgt_in_w, tgt_in_b, tgt_out_w, tgt_out_b)]

    in_maps = []
    scatter = []
    for c in range(8):
        b, role = c // 2, c % 2
        q_idx = ctx_idx[b] if role == 0 else tgt_idx[b]
        kv_idx = ctx_idx[b]
        wq, wk, wv, wo, bvb, bob, biasqk = packs[role]
        in_maps.append({
            "xqT": _pmajor(np.ascontiguousarray(xr[b][q_idx].T)),
            "xkT": _pmajor(np.ascontiguousarray(xr[b][kv_idx].T)),
            "wq": wq, "wk": wk, "wv": wv, "wo": wo,
            "bvb": bvb, "bob": bob, "biasqk": biasqk,
        })
        scatter.append((b, q_idx))
    return in_maps, scatter


_NC_CACHE = None


def kernel(**inputs):
    global _NC_CACHE
    in_maps, scatter = host_prep(**inputs)
    if _NC_CACHE is None:
        _NC_CACHE = build_nc()
    nc = _NC_CACHE
    res = run_bass_kernel_spmd(nc, in_maps, core_ids=list(range(8)))
    x = np.asarray(inputs["x"], np.float32)
    out = np.zeros_like(x)
    for c in range(8):
        b, q_idx = scatter[c]
        out[b][q_idx] = res.results[c]["out"]
    return out


# revision 18
# speedup vs baseline: 1.0678x; 1.0202x over previous
"""Trainium2 Bass kernel for nn_CNNCrossPatchBackbone (sparse cross-patch attention).

Strategy: 8 cores = 4 batches x {ctx self-attention, tgt cross-attention}.
Fully task-parallel, no collectives. Each core: 1024 q-tokens x 1024
kv-tokens, 16 heads of dim 64, D=1024.

v3.5 vs v3.3 (308.7us):
  * All 1-partition bias matmuls removed: V bias and out-proj bias applied
    by the DVE during PSUM evacuation (tensor_add against host-prebroadcast
    bias tiles). Saves ~10us of PE at 317ns per bias matmul.
  * Q-projection chains 1..7 moved INTO the phase-2 head loop (one chain
    every other head), filling the PE slack under the ACT exp floor; Q bias
    applied via DVE tensor_scalar_add (per-partition scalar). Shrinks the
    serial phase-1 region by ~24us.
  * Phase 2 per head: 8 uniform 1024-wide exp chunks (shared 3-buf psum),
    AV into two single-bank [65,512] psums (full-speed 216ns AV matmuls;
    the 2-bank [65,1024] variant ran at 334ns), denominator reciprocal on
    SBUF copy (custom DVE ops corrupt on PSUM input), GPS broadcast +
    mixed-dtype normalize-mul lag one head behind, off the critical path.

Measured engine floors per core: ACT exp 8.5us/head (phase-2 pacer), PE
~216ns/512-col matmul when uncoupled. Predicted ~275us.
"""

import sys

sys.path.insert(0, "/opt/trn_rl_repo")

import ml_dtypes
import numpy as np

import concourse.bass as bass  # noqa: F401
import concourse.tile as tile
from concourse import bacc, mybir
from concourse.bass_utils import run_bass_kernel_spmd

B, K, D, H = 4, 2048, 1024, 16
NCTX = K // 2
NTOK = 1024
HD = D // H  # 64
IMAGE_SIZE = 224.0
MAX_POS = 1024
P = 128
DT = D // P  # 8
TT = NTOK // P  # 8
F32 = mybir.dt.float32
BF16 = mybir.dt.bfloat16
BF = ml_dtypes.bfloat16


def build_nc():
    nc = bacc.Bacc("TRN2", target_bir_lowering=False, debug=False, num_devices=8)

    xqT_ext = nc.dram_tensor("xqT", [P, DT * NTOK], BF16, kind="ExternalInput")
    xkT_ext = nc.dram_tensor("xkT", [P, DT * NTOK], BF16, kind="ExternalInput")
    wq_ext = nc.dram_tensor("wq", [P, DT * D], BF16, kind="ExternalInput")
    wk_ext = nc.dram_tensor("wk", [P, DT * D], BF16, kind="ExternalInput")
    wv_ext = nc.dram_tensor("wv", [P, DT * D], BF16, kind="ExternalInput")
    wo_ext = nc.dram_tensor("wo", [P, DT * D], BF16, kind="ExternalInput")
    bvb_ext = nc.dram_tensor("bvb", [P, D], BF16, kind="ExternalInput")
    bob_ext = nc.dram_tensor("bob", [P, D], F32, kind="ExternalInput")
    biasqk_ext = nc.dram_tensor("biasqk", [P, 2 * DT], F32, kind="ExternalInput")
    out_ext = nc.dram_tensor("out", [NTOK, D], F32, kind="ExternalOutput")

    from contextlib import ExitStack

    with tile.TileContext(nc) as tc:
        es0 = ExitStack()   # whole-kernel sbuf pools
        psA = ExitStack()   # shared [128,1024]f32 psum (proj chains + S chunks)
        esW = ExitStack()   # wo staging (lives to end)
        esK = ExitStack()   # wv/xk/wq/xq staging (live through phase 2)
        es1a = ExitStack()  # wk staging (closes after K chains)
        es2 = ExitStack()   # phase-2 sbuf pools (open after K chains)
        es3 = ExitStack()   # phase-2 AV psum
        es4 = ExitStack()   # phase-3 pools

        cpool = es0.enter_context(tc.tile_pool(name="const", bufs=1))
        p_qt = es0.enter_context(tc.tile_pool(name="p_qt", bufs=3))
        p_spr = es0.enter_context(tc.tile_pool(name="p_spr", bufs=H))
        p_va = es0.enter_context(tc.tile_pool(name="p_va", bufs=TT))
        p_ot = es0.enter_context(tc.tile_pool(name="p_ot", bufs=DT))

        ones_f = cpool.tile([P, P], F32)
        nc.gpsimd.memset(ones_f[:], 1.0)
        ones_bf = cpool.tile([P, P], BF16)
        nc.vector.tensor_copy(ones_bf[:], ones_f[:])
        biasT = cpool.tile([P, 2 * DT], F32)
        nc.sync.dma_start(biasT[:], biasqk_ext.ap())
        bvb = cpool.tile([P, D], BF16)
        nc.scalar.dma_start(bvb[:], bvb_ext.ap())
        bob = cpool.tile([P, D], F32)
        nc.scalar.dma_start(bob[:], bob_ext.ap())

        QT = {}  # chain idx -> [128, NTOK] bf16 tile
        # SPR[h]: K^T for head h in rows (h%2)*64..+64, other rows ZERO so the
        # S matmul can contract all 128 partitions (uniform 128-part
        # stationaries keep PE kind-transitions cheap: 64<->128 partition
        # switches cost ~350ns/matmul, measured)
        SPR = [p_spr.tile([P, NTOK], BF16, tag="spr", name=f"spr{h}") for h in range(H)]
        VA = [p_va.tile([P, H * (HD + 1)], BF16, tag="va", name=f"va{i}") for i in range(TT)]
        OT = [p_ot.tile([P, NTOK], BF16, tag="ot", name=f"ot{i}") for i in range(DT)]
        for va in VA:
            nc.vector.tensor_copy(
                va[:].rearrange("p (h c) -> p h c", c=HD + 1)[:, :, HD : HD + 1],
                ones_bf[:, 0:H].rearrange("p (h c) -> p h c", c=1),
            )

        # ---- staging + DMAs ----
        ps = psA.enter_context(tc.tile_pool(name="ps", bufs=3, space="PSUM"))
        p_wo = esW.enter_context(tc.tile_pool(name="p_wo", bufs=1))
        p_wv = esK.enter_context(tc.tile_pool(name="p_wv", bufs=1))
        p_xk = esK.enter_context(tc.tile_pool(name="p_xk", bufs=1))
        p_wq = esK.enter_context(tc.tile_pool(name="p_wq", bufs=1))
        p_xq = esK.enter_context(tc.tile_pool(name="p_xq", bufs=1))
        p_wk = es1a.enter_context(tc.tile_pool(name="p_wk", bufs=1))

        def stage(pool, n, tag):
            t = pool.tile([P, n * NTOK], BF16, tag=tag, name=tag)
            return t, [t[:, i * NTOK : (i + 1) * NTOK] for i in range(n)]

        # wk/xk split per-dt on two queues: K chains start as tiles land.
        wk_t, WK = stage(p_wk, DT, "wk")
        xk_t, XK = stage(p_xk, DT, "xk")
        for dt in range(DT):
            nc.sync.dma_start(
                wk_t[:, dt * NTOK : (dt + 1) * NTOK],
                wk_ext.ap()[:, dt * NTOK : (dt + 1) * NTOK],
            )
            nc.gpsimd.dma_start(
                xk_t[:, dt * NTOK : (dt + 1) * NTOK],
                xkT_ext.ap()[:, dt * NTOK : (dt + 1) * NTOK],
            )
        wq_t, WQ = stage(p_wq, DT, "wq")
        nc.sync.dma_start(wq_t[:], wq_ext.ap())
        xq_t, XQ = stage(p_xq, DT, "xq")
        nc.gpsimd.dma_start(xq_t[:], xqT_ext.ap())
        wv_t, WV = stage(p_wv, DT, "wv")
        nc.scalar.dma_start(wv_t[:], wv_ext.ap())
        wo_t, WO = stage(p_wo, DT, "wo")
        nc.sync.dma_start(wo_t[:], wo_ext.ap())
        # zero the unused half of each SPR tile (GPS is idle during the DMAs)
        for h in range(H):
            po = (h % 2) * HD
            nc.gpsimd.memset(SPR[h][HD - po : P - po, :], 0.0)

        # ---- phase 1: K chains + Q chain 0 (ACT identity evac) ----
        for c in range(DT):
            kps = ps.tile([P, NTOK], F32, tag="ps", name=f"kps{c}")
            for nh in range(2):
                sl = slice(nh * 512, (nh + 1) * 512)
                for dt in range(DT):
                    nc.tensor.matmul(
                        kps[:, sl], WK[dt][:, c * P : (c + 1) * P], XK[dt][:, sl],
                        start=(dt == 0), stop=(dt == DT - 1),
                    )
            nc.scalar.activation(
                SPR[2 * c][0:HD, :], kps[0:HD, :],
                mybir.ActivationFunctionType.Identity,
                bias=biasT[0:HD, DT + c : DT + c + 1],
            )
            nc.scalar.activation(
                SPR[2 * c + 1][HD:P, :], kps[HD:P, :],
                mybir.ActivationFunctionType.Identity,
                bias=biasT[HD:P, DT + c : DT + c + 1],
            )

        def emit_q_chain(c, evac_dve):
            qps = ps.tile([P, NTOK], F32, tag="ps", name=f"qps{c}")
            for nh in range(2):
                sl = slice(nh * 512, (nh + 1) * 512)
                for dt in range(DT):
                    nc.tensor.matmul(
                        qps[:, sl], WQ[dt][:, c * P : (c + 1) * P], XQ[dt][:, sl],
                        start=(dt == 0), stop=(dt == DT - 1),
                    )
            QT[c] = p_qt.tile([P, NTOK], BF16, tag="qt", name=f"qt{c}")
            if evac_dve:
                nc.vector.tensor_scalar_add(QT[c][:], qps[:], biasT[:, c : c + 1])
            else:
                nc.scalar.activation(
                    QT[c][:], qps[:],
                    mybir.ActivationFunctionType.Identity,
                    bias=biasT[:, c : c + 1],
                )

        emit_q_chain(0, evac_dve=False)
        es1a.close()  # wk staging done

        # ---- phase-2 sbuf pools ----
        p_a = es2.enter_context(tc.tile_pool(name="p_a", bufs=14))
        p_rdn = es2.enter_context(tc.tile_pool(name="p_rdn", bufs=1))
        p_rr = es2.enter_context(tc.tile_pool(name="p_rr", bufs=1))
        p_rb = es2.enter_context(tc.tile_pool(name="p_rb", bufs=2))
        p_os = es2.enter_context(tc.tile_pool(name="p_os", bufs=2))

        AT = {}  # (h, kc) -> a tile [128k, 1024q] bf16

        def emit_s_chunk(h, kc):
            qt, po = h // 2, (h % 2) * HD
            s_ps = ps.tile([P, NTOK], F32, tag="ps", name=f"s{h}_{kc}")
            for j in range(2):
                nc.tensor.matmul(
                    s_ps[:, j * 512 : (j + 1) * 512],
                    SPR[h][:, kc * P : (kc + 1) * P],
                    QT[qt][:, j * 512 : (j + 1) * 512],
                    start=True, stop=True,
                )
            a_t = p_a.tile([P, NTOK], BF16, tag="a", name=f"a{h}_{kc}")
            nc.scalar.activation(a_t[:], s_ps[:], mybir.ActivationFunctionType.Exp)
            AT[(h, kc)] = a_t

        # V chains (DVE bias-add evac) interleaved with S/exp of head 0
        for tt in range(TT):
            vps = ps.tile([P, NTOK], F32, tag="ps", name=f"vps{tt}")
            for nh in range(2):
                sl = slice(nh * 512, (nh + 1) * 512)
                for dt in range(DT):
                    nc.tensor.matmul(
                        vps[:, sl], XK[dt][:, tt * P : (tt + 1) * P],
                        WV[dt][:, sl],
                        start=(dt == 0), stop=(dt == DT - 1),
                    )
            nc.vector.tensor_add(
                VA[tt][:].rearrange("p (h c) -> p h c", c=HD + 1)[:, :, 0:HD],
                bvb[:].rearrange("p (h c) -> p h c", c=HD),
                vps[:].rearrange("p (h c) -> p h c", c=HD),
            )
            emit_s_chunk(0, tt)

        ps_o = es3.enter_context(tc.tile_pool(name="ps_o", bufs=2, space="PSUM"))

        # ---- phase 2 main loop: S(i+1) | AV(i) | Q-chain | norm(i) | mul(i-1)
        def emit_av(ha, o_ps, qh):
            for kc in range(TT):
                nc.tensor.matmul(
                    o_ps[:],
                    VA[kc][:, ha * (HD + 1) : (ha + 1) * (HD + 1)],
                    AT[(ha, kc)][:, qh * 512 : (qh + 1) * 512],
                    start=(kc == 0), stop=(kc == TT - 1),
                )

        state = {}  # ha -> (rb, o_scr)
        for i in range(H + 1):
            hs, ha = i + 1, i
            if hs < H:
                emit_s_chunk(hs, 0)
                emit_s_chunk(hs, 1)
            if ha < H:
                o_ps0 = ps_o.tile([HD + 1, 512], F32, tag="o", name=f"o{ha}_0")
                emit_av(ha, o_ps0, 0)
            if hs < H:
                emit_s_chunk(hs, 2)
                emit_s_chunk(hs, 3)
            if ha < H:
                o_ps1 = ps_o.tile([HD + 1, 512], F32, tag="o", name=f"o{ha}_1")
                emit_av(ha, o_ps1, 1)
            if hs < H:
                emit_s_chunk(hs, 4)
                emit_s_chunk(hs, 5)
            if ha < H:
                # custom DVE ops need SBUF input (PSUM reads silently corrupt
                # on HW) — copy the denominator rows out first
                dn_t = p_rdn.tile([1, NTOK], F32, tag="dn", name=f"dn{ha}")
                nc.vector.tensor_copy(dn_t[:, 0:512], o_ps0[HD : HD + 1, :])
                nc.vector.tensor_copy(dn_t[:, 512:1024], o_ps1[HD : HD + 1, :])
                r_t = p_rr.tile([1, NTOK], F32, tag="r", name=f"r{ha}")
                nc.vector.reciprocal_approx_fast(r_t[:], dn_t[:])
                rb = p_rb.tile([HD, NTOK], F32, tag="rb", name=f"rb{ha}")
                nc.gpsimd.partition_broadcast(rb[:], r_t[:], channels=HD)
            qc = i // 2 + 1
            if i % 2 == 0 and qc < DT:
                emit_q_chain(qc, evac_dve=True)
            if hs < H:
                emit_s_chunk(hs, 6)
                emit_s_chunk(hs, 7)
            if ha < H:
                o_scr = p_os.tile([HD, NTOK], BF16, tag="os", name=f"os{ha}")
                nc.vector.tensor_copy(o_scr[:, 0:512], o_ps0[0:HD, :])
                nc.vector.tensor_copy(o_scr[:, 512:1024], o_ps1[0:HD, :])
                state[ha] = (rb, o_scr)
            hm = i - 1
            if 0 <= hm < H:
                rb_m, os_m = state.pop(hm)
                qt, po = hm // 2, (hm % 2) * HD
                nc.vector.tensor_mul(OT[qt][po : po + HD, :], os_m[:], rb_m[:])
                for kc in range(TT):
                    del AT[(hm, kc)]

        es3.close()  # ps_o
        psA.close()  # shared psum -> 8 banks free for ps_y
        es2.close()
        esK.close()

        # ---- phase 3: output projection (DVE bias-add evac) ----
        p_y = es4.enter_context(tc.tile_pool(name="p_y", bufs=4))
        ps_y = es4.enter_context(tc.tile_pool(name="ps_y", bufs=3, space="PSUM"))
        for qc in range(TT):
            y_ps = ps_y.tile([P, D], F32, tag="y", name=f"yps{qc}")
            for nh in range(2):
                sl = slice(nh * 512, (nh + 1) * 512)
                for dt in range(DT):
                    nc.tensor.matmul(
                        y_ps[:, sl], OT[dt][:, qc * P : (qc + 1) * P],
                        WO[dt][:, sl],
                        start=(dt == 0), stop=(dt == DT - 1),
                    )
            y_t = p_y.tile([P, D], F32, tag="yt", name=f"yt{qc}")
            nc.vector.tensor_add(y_t[:], y_ps[:], bob[:])
            nc.sync.dma_start(out_ext.ap()[qc * P : (qc + 1) * P, :], y_t[:])
        es4.close()
        esW.close()
        es0.close()

    nc.compile()
    return nc


# ---------------------------------------------------------------------------
# host side
# ---------------------------------------------------------------------------

def _pmajor(a):
    """[DT*P, N] -> [P, DT*N] partition-major bf16 (contiguous 16KB rows)."""
    d, n = a.shape
    return np.ascontiguousarray(
        a.reshape(DT, P, n).transpose(1, 0, 2).reshape(P, DT * n)
    ).astype(BF)


def host_prep(x, coords, is_context, rope_cache,
              ctx_in_w, ctx_in_b, ctx_out_w, ctx_out_b,
              tgt_in_w, tgt_in_b, tgt_out_w, tgt_out_b):
    x = np.asarray(x, np.float32)
    coords = np.asarray(coords, np.float32)
    is_context = np.asarray(is_context, bool)
    rope_cache = np.asarray(rope_cache, np.float32)

    keys = np.where(is_context, 0, 1).astype(np.int32)
    order = np.argsort(keys, axis=1, kind="stable")
    ctx_idx = order[:, :NCTX]
    tgt_idx = order[:, NCTX:]

    # rope rotation (mirrors reference fp32 arithmetic)
    cn = np.clip(
        coords / np.float32(IMAGE_SIZE) * np.float32(MAX_POS - 1), 0, MAX_POS - 1
    )
    y_pos = cn[..., 0].astype(np.int32)
    x_pos = cn[..., 1].astype(np.int32)
    cx = rope_cache[x_pos, :, 0]
    sx = rope_cache[x_pos, :, 1]
    cy = rope_cache[y_pos, :, 0]
    sy = rope_cache[y_pos, :, 1]
    half = D // 2
    xr = np.empty_like(x)
    xe = x[:, :, 0:half:2]
    xo = x[:, :, 1:half:2]
    xr[:, :, 0:half:2] = xe * cx - xo * sx
    xr[:, :, 1:half:2] = xe * sx + xo * cx
    ye = x[:, :, half::2]
    yo = x[:, :, half + 1 :: 2]
    xr[:, :, half::2] = ye * cy - yo * sy
    xr[:, :, half + 1 :: 2] = ye * sy + yo * cy

    def pack_w(in_w, in_b, out_w, out_b):
        w = np.array(in_w, np.float32)
        b3 = np.array(in_b, np.float32).copy()
        w[0:D] *= np.float32(0.125)  # fold 1/sqrt(hd) into Wq
        b3[0:D] *= np.float32(0.125)
        wT = np.ascontiguousarray(w.T)  # [D, 3D]
        wq = _pmajor(np.ascontiguousarray(wT[:, 0:D]))
        wk = _pmajor(np.ascontiguousarray(wT[:, D : 2 * D]))
        wv = _pmajor(np.ascontiguousarray(wT[:, 2 * D :]))
        wo = _pmajor(np.ascontiguousarray(np.asarray(out_w, np.float32).T))
        bvb = np.broadcast_to(b3[None, 2 * D :], (P, D)).astype(BF)
        bob = np.ascontiguousarray(
            np.broadcast_to(np.asarray(out_b, np.float32)[None, :], (P, D))
        )
        biasqk = np.zeros((P, 2 * DT), np.float32)
        biasqk[:, 0:DT] = b3[0:D].reshape(DT, P).T
        biasqk[:, DT:] = b3[D : 2 * D].reshape(DT, P).T
        return wq, wk, wv, wo, bvb, bob, biasqk

    packs = [pack_w(ctx_in_w, ctx_in_b, ctx_out_w, ctx_out_b),
             pack_w(tgt_in_w, tgt_in_b, tgt_out_w, tgt_out_b)]

    in_maps = []
    scatter = []
    for c in range(8):
        b, role = c // 2, c % 2
        q_idx = ctx_idx[b] if role == 0 else tgt_idx[b]
        kv_idx = ctx_idx[b]
        wq, wk, wv, wo, bvb, bob, biasqk = packs[role]
        in_maps.append({
            "xqT": _pmajor(np.ascontiguousarray(xr[b][q_idx].T)),
            "xkT": _pmajor(np.ascontiguousarray(xr[b][kv_idx].T)),
            "wq": wq, "wk": wk, "wv": wv, "wo": wo,
            "bvb": bvb, "bob": bob, "biasqk": biasqk,
        })
        scatter.append((b, q_idx))
    return in_maps, scatter


_NC_CACHE = None


def kernel(**inputs):
    global _NC_CACHE
    in_maps, scatter = host_prep(**inputs)
    if _NC_CACHE is None:
        _NC_CACHE = build_nc()
    nc = _NC_CACHE
    res = run_bass_kernel_spmd(nc, in_maps, core_ids=list(range(8)))
    x = np.asarray(inputs["x"], np.float32)
    out = np.zeros_like(x)
    for c in range(8):
        b, q_idx = scatter[c]
        out[b][q_idx] = res.results[c]["out"]
    return out
